# revision 1
# baseline (speedup 1.0000x reference)
"""EmergentVision Trainium2 kernel: conv frontend + 12-step spiking RNN + classifier.

Strategy (8 NeuronCores):
  - Vision frontend: data-parallel over batch (32 images/core), convs as
    tap-packed matmuls, then AllGather of the flattened features.
  - Recurrent GEMM: tensor-parallel over the 4096 output dim. Each core holds
    eff_w[j_shard, :].T (4096x512, fp32) resident in SBUF, masked on device.
    Per step: GEMM in float32r (full-rate fp32 path at N>=256), LayerNorm
    stats via ones-matmul partition reduction + AllGather of the 2x256
    partials, fused neuron update, AllGather of transposed spikes.
  - Classifier: replicated small matmul after the last spike AllGather.
"""

import numpy as np

N = 4096
B = 256
NCORES = 8
JS = N // NCORES        # 512 j-shard per core
BS = B // NCORES        # 32 images per core
JT = JS // 128          # 4 j-tiles of 128
KC = N // 128           # 32 contraction chunks
STEPS = 12
LEAK = 0.95
EPS = 1e-5
CONN_THR = 0.01
CDIM = 64 * 7 * 7       # 3136
CKC = 25                # ceil(3136/128); last chunk has 64 rows

_PROGRAM_CACHE = {}


def _build_program(steps=STEPS, skip_stats=False, skip_spike_ag=False,
                   skip_gemm=False, skip_elem=False):
    import concourse.bass as bass
    import concourse.mybir as mybir
    import concourse.tile as tile
    from concourse import bacc
    from concourse.bass import ts, ds
    from concourse.masks import make_identity
    from contextlib import ExitStack

    f32 = mybir.dt.float32
    f32r = mybir.dt.float32r
    Alu = mybir.AluOpType
    Act = mybir.ActivationFunctionType

    def R(ap):
        return ap.bitcast(f32r)

    def F(ap):
        return ap.bitcast(f32)

    nc = bacc.Bacc("TRN2", target_bir_lowering=False, debug=False,
                   num_devices=NCORES)

    # ---------------- inputs ----------------
    xpad_d = nc.dram_tensor("xpad", [BS, 30, 30], f32, kind="ExternalInput").ap()
    w1t_d = nc.dram_tensor("w1t", [9, 32], f32, kind="ExternalInput").ap()
    w2col_d = nc.dram_tensor("w2col", [3, 96, 64], f32r, kind="ExternalInput").ap()
    bn1_d = nc.dram_tensor("bn1", [4, 32], f32, kind="ExternalInput").ap()
    bn2_d = nc.dram_tensor("bn2", [4, 64], f32, kind="ExternalInput").ap()
    fcwT_d = nc.dram_tensor("fcwT", [CDIM, JS], f32r, kind="ExternalInput").ap()
    fcb_d = nc.dram_tensor("fcb", [JS], f32, kind="ExternalInput").ap()
    wT_d = nc.dram_tensor("wT", [N, JS], f32r, kind="ExternalInput").ap()
    lng_d = nc.dram_tensor("lng", [JS], f32, kind="ExternalInput").ap()
    lnb_d = nc.dram_tensor("lnb", [JS], f32, kind="ExternalInput").ap()
    thr_d = nc.dram_tensor("thr", [JS], f32, kind="ExternalInput").ap()
    intr_d = nc.dram_tensor("intr", [JS], f32, kind="ExternalInput").ap()
    clswT_d = nc.dram_tensor("clswT", [N, 10], f32r, kind="ExternalInput").ap()
    clsb_d = nc.dram_tensor("clsb", [10], f32, kind="ExternalInput").ap()
    out_d = nc.dram_tensor("out", [10, B], f32, kind="ExternalOutput").ap()

    RG = [list(range(NCORES))]

    with tile.TileContext(nc) as tc, ExitStack() as st:
        st.enter_context(nc.allow_low_precision(
            reason="fp32r (11-bit-mantissa fp32) matmul inputs are intentional"))
        const = st.enter_context(tc.tile_pool(name="const", bufs=1))
        work = st.enter_context(tc.tile_pool(name="work", bufs=2))
        dram = st.enter_context(tc.tile_pool(name="dram", bufs=2, space="DRAM"))
        psum = st.enter_context(tc.tile_pool(name="psum", bufs=4, space="PSUM"))

        # ---------------- persistent SBUF ----------------
        w_sb = const.tile([128, KC, JS], f32r)         # 8MB eff_w.T shard
        spT = const.tile([128, KC, B], f32r)           # 4MB gathered spikes.T
        xc = const.tile([128, JT, B], f32r)            # LN input copy
        q_sb = const.tile([128, JT, B], f32)          # neuron state min(g,0)
        s_sb = const.tile([128, JT, B], f32r)          # spikes shard
        ones = const.tile([128, 128], f32r)
        ident = const.tile([128, 128], f32)
        lng_sb = const.tile([128, JT], f32)
        thr_sb = const.tile([128, JT], f32)
        cb_sb = const.tile([128, JT], f32)            # lnb + intr + thr*(LEAK-1)
        fcb_sb = const.tile([128, JT], f32)
        clsw_sb = const.tile([128, KC, 10], f32r)
        clsb_sb = const.tile([10, 1], f32)

        zeros_f = const.tile([128, JS], f32)
        nc.gpsimd.memset(zeros_f[:], 0.0)
        ones_f = work.tile([128, 128], f32, tag="onesf", bufs=1)
        nc.gpsimd.memset(ones_f[:], 1.0)
        nc.vector.tensor_copy(ones[:], ones_f[:])
        make_identity(nc, ident[:])

        # ---------------- load + prep params ----------------
        nc.sync.dma_start(lng_sb[:], lng_d.rearrange("(t p) -> p t", p=128))
        nc.sync.dma_start(thr_sb[:], thr_d.rearrange("(t p) -> p t", p=128))
        lnb_t = work.tile([128, JT], f32, tag="ptmp")
        intr_t = work.tile([128, JT], f32, tag="ptmp")
        nc.sync.dma_start(lnb_t[:], lnb_d.rearrange("(t p) -> p t", p=128))
        nc.sync.dma_start(intr_t[:], intr_d.rearrange("(t p) -> p t", p=128))
        nc.vector.tensor_scalar_mul(cb_sb[:], thr_sb[:], LEAK - 1.0)
        nc.vector.tensor_tensor(cb_sb[:], cb_sb[:], intr_t[:], Alu.add)
        nc.vector.tensor_tensor(cb_sb[:], cb_sb[:], lnb_t[:], Alu.add)
        nc.sync.dma_start(fcb_sb[:], fcb_d.rearrange("(t p) -> p t", p=128))
        nc.sync.dma_start(clsw_sb[:], clswT_d.rearrange("(c p) o -> p c o", p=128))
        nc.sync.dma_start(clsb_sb[:], clsb_d[:, None])

        # q init: q = -thr (broadcast over free dim)
        nc.gpsimd.memset(q_sb[:], 0.0)
        for jt in range(JT):
            nc.gpsimd.tensor_scalar(q_sb[:, jt, :], q_sb[:, jt, :],
                                    thr_sb[:, jt:jt + 1], None, Alu.subtract)

        # ---------------- recurrent weights: load + mask ----------------
        nc.sync.dma_start(w_sb[:], wT_d.rearrange("(c p) j -> p c j", p=128))
        for c in range(KC):
            wa = work.tile([128, JS], f32, tag="wabs")
            wm = work.tile([128, JS], f32, tag="wmask")
            nc.scalar.activation(wa[:], F(w_sb[:, c, :]), Act.Abs)
            nc.gpsimd.tensor_scalar(wm[:], wa[:], CONN_THR, None, Alu.is_gt)
            nc.vector.tensor_tensor(w_sb[:, c, :], F(w_sb[:, c, :]), wm[:],
                                    Alu.mult)

        # ---------------- bn params ----------------
        def bn_prep(bn_d, nchan, nrep):
            p = nchan * nrep
            raw = const.tile([p, 4], f32, name=f"bnraw{nchan}")
            for rep in range(nrep):
                nc.sync.dma_start(raw[rep * nchan:(rep + 1) * nchan, :],
                                  bn_d.rearrange("f c -> c f"))
            s_t = const.tile([p, 1], f32, name=f"bns{nchan}")
            t_t = const.tile([p, 1], f32, name=f"bnt{nchan}")
            tmp = work.tile([p, 1], f32, tag="bntmp")
            # s = g / sqrt(v + eps); t = b - m*s
            nc.vector.tensor_scalar_add(tmp[:], raw[:, 3:4], EPS)
            nc.scalar.activation(tmp[:], tmp[:], Act.Sqrt)
            nc.vector.reciprocal(s_t[:], tmp[:])
            nc.vector.tensor_tensor(s_t[:], s_t[:], raw[:, 0:1], Alu.mult)
            nc.vector.tensor_tensor(tmp[:], raw[:, 2:3], s_t[:], Alu.mult)
            nc.vector.tensor_tensor(t_t[:], raw[:, 1:2], tmp[:], Alu.subtract)
            return s_t, t_t

        bn1s, bn1t = bn_prep(bn1_d, 32, 4)
        bn2s, bn2t = bn_prep(bn2_d, 64, 1)

        # ---------------- conv frontend ----------------
        x2pad4 = None
        h_loc_d = dram.tile([BS, CDIM], f32, bufs=1)

        with tc.tile_pool(name="convA", bufs=1) as convA:
            x2pad4 = convA.tile([128, 8, 16, 16], f32r)
            nc.gpsimd.memset(F(x2pad4[:]), 0.0)
            h_sb = convA.tile([64, BS, 49], f32)

            with tc.tile_pool(name="conv1", bufs=1) as conv1p:
                rhs9 = conv1p.tile([128, 6272], f32)
                act1 = rhs9  # chunk columns are dead after their matmuls
                w1rep = conv1p.tile([128, 32], f32)
                for bq in range(4):
                    nc.sync.dma_start(w1rep[32 * bq:32 * bq + 9, :], w1t_d[:])
                    for t in range(9):
                        dy, dx = t // 3, t % 3
                        dst = rhs9[32 * bq + t:32 * bq + t + 1, :].rearrange(
                            "p (b i j) -> p b i j", b=8, i=28, j=28)
                        nc.sync.dma_start(
                            dst, xpad_d[bq * 8:(bq + 1) * 8,
                                        dy:dy + 28, dx:dx + 28][None])
                # conv1 matmuls: 4 col/row-tiled strips, 13 chunks each
                offs = [(i * 512, 512) for i in range(12)] + [(6144, 128)]
                for (off, cw) in offs:
                    ps1 = psum.tile([128, 512], f32, tag="gemm", name="ps1")
                    for bq in range(4):
                        nc.tensor.matmul(
                            ps1[32 * bq:32 * bq + 32, :cw],
                            w1rep[32 * bq:32 * bq + 9, :],
                            rhs9[32 * bq:32 * bq + 9, ds(off, cw)],
                            start=True, stop=True,
                            tile_position=(32 * bq, 32 * bq))
                    nc.scalar.activation(act1[:, ds(off, cw)], ps1[:, :cw],
                                         Act.Relu, bias=bn1t[:], scale=bn1s[:])
                # maxpool 2x2 -> write into padded x2pad4 interior
                av = act1.rearrange("p (b i2 iw j2 jw) -> p b i2 iw j2 jw",
                                    b=8, i2=14, iw=2, j2=14, jw=2)
                m1 = conv1p.tile([128, 8, 14, 14], f32)
                nc.any.tensor_tensor(m1[:], av[:, :, :, 0, :, 0],
                                     av[:, :, :, 0, :, 1], Alu.max)
                nc.any.tensor_tensor(x2pad4[:, :, 1:15, 1:15],
                                     av[:, :, :, 1, :, 0],
                                     av[:, :, :, 1, :, 1], Alu.max)
                nc.any.tensor_tensor(x2pad4[:, :, 1:15, 1:15],
                                     F(x2pad4[:, :, 1:15, 1:15]), m1[:],
                                     Alu.max)

            with tc.tile_pool(name="conv2", bufs=1) as conv2p:
                w2_sb = conv2p.tile([128, 3, 64], f32r)
                nc.sync.dma_start(w2_sb[0:96, :, :],
                                  w2col_d.rearrange("d p o -> p d o"))
                hv = h_sb.rearrange("p b (i j) -> p b i j", i=7, j=7)
                for bh in range(2):  # two halves of 16 images
                    x2col = conv2p.tile([128, 16, 16, 16], f32r, tag="x2col")
                    act2 = conv2p.tile([64, 16 * 196], f32, tag="act2")
                    for tx in range(3):
                        for bq in range(2 * bh, 2 * bh + 2):
                            bo = (bq - 2 * bh) * 8
                            nc.sync.dma_start(
                                x2col[32 * tx:32 * tx + 32,
                                      bo:bo + 8, :, 0:16 - tx],
                                x2pad4[32 * bq:32 * bq + 32, :, :, tx:16])
                    for ch in range(8):
                        ps2 = psum.tile([64, 392], f32, tag="gemm", name="ps2")
                        for dy in range(3):
                            nc.tensor.matmul(
                                ps2[:],
                                R(w2_sb[0:96, dy, :]),
                                R(x2col[0:96, 2 * ch:2 * ch + 2,
                                        dy:dy + 14, 0:14]),
                                start=(dy == 0), stop=(dy == 2))
                        nc.scalar.activation(
                            act2[:, ds(ch * 392, 392)], ps2[:], Act.Relu,
                            bias=bn2t[:], scale=bn2s[:])
                    # maxpool 2x2 -> h [64, 16, 7, 7] for this half
                    av2 = act2.rearrange(
                        "p (b i2 iw j2 jw) -> p b i2 iw j2 jw",
                        b=16, i2=7, iw=2, j2=7, jw=2)
                    n1 = conv2p.tile([64, 16, 7, 7], f32, tag="n1")
                    hvh = hv[:, bh * 16:(bh + 1) * 16, :, :]
                    nc.any.tensor_tensor(n1[:], av2[:, :, :, 0, :, 0],
                                         av2[:, :, :, 0, :, 1], Alu.max)
                    nc.any.tensor_tensor(hvh, av2[:, :, :, 1, :, 0],
                                         av2[:, :, :, 1, :, 1], Alu.max)
                    nc.any.tensor_tensor(hvh, hvh, n1[:], Alu.max)

            # h -> DRAM as [b, c] with c = oc*49 + ij
            nc.sync.dma_start(
                h_loc_d.rearrange("b (oc ij) -> oc b ij", oc=64), h_sb[:])

        # AllGather h across cores -> [256, 3136]
        h_all_d = dram.tile([B, CDIM], f32, bufs=1, addr_space="Shared")
        nc.gpsimd.collective_compute(
            "AllGather", Alu.bypass, replica_groups=RG,
            ins=[h_loc_d[:].opt()], outs=[h_all_d[:].opt()])

        # ---------------- transpose h, fc GEMM ----------------
        psum_xs = [psum.tile([128, B], f32, tag="gemm", name=f"px{j}")
                   for j in range(JT)]
        with tc.tile_pool(name="fcp", bufs=1) as fcp:
            hT = fcp.tile([128, CKC, B], f32r)
            nc.vector.tensor_copy(hT[64:, CKC - 1, :], zeros_f[64:, 0:B])
            for bt in range(2):
                hall = fcp.tile([128, CDIM], f32, tag="hall", bufs=1)
                nc.sync.dma_start(hall[:], h_all_d[bt * 128:(bt + 1) * 128, :])
                for cc in range(CKC):
                    cw = 128 if cc < CKC - 1 else 64
                    pt = psum.tile([128, 128], f32, tag="bc", bufs=2, name="ptr")
                    nc.tensor.transpose(pt[:cw, :], hall[:, ds(cc * 128, cw)],
                                        ident[:])
                    nc.any.tensor_copy(hT[:cw, cc, ts(bt, 128)], pt[:cw, :])
            quarters = [list(range(5 * i, 5 * i + 5)) for i in range(5)]
            fv = fcwT_d[0:3072, :].rearrange("(c p) j -> p c j", p=128, c=24)
            for qi, kcs in enumerate(quarters):
                fq = fcp.tile([128, 5, JS], f32r, tag="fcw", bufs=2, name="fq")
                nfull = len([k for k in kcs if k < 24])
                nc.sync.dma_start(fq[:, 0:nfull, :], fv[:, kcs[0]:kcs[0] + nfull, :])
                if kcs[-1] == 24:
                    nc.vector.tensor_copy(fq[64:, nfull, :], zeros_f[64:, :])
                    nc.sync.dma_start(fq[0:64, nfull, :],
                                      fcwT_d[3072:3136, :][:])
                for ki, kc in enumerate(kcs):
                    for jt in range(JT):
                        nc.tensor.matmul(
                            psum_xs[jt][:],
                            R(fq[:, ki, ts(jt, 128)]),
                            R(hT[:, kc, :]),
                            start=(kc == 0), stop=(kc == CKC - 1))

        # ---------------- recurrent steps ----------------
        def stats_and_elementwise(step, pxs):
            # copy GEMM out to SBUF (+fc bias on step 0), squares, partial sums
            ps_sx = psum.tile([1, B], f32, tag="stats", bufs=2, name="ps_sx")
            ps_sxx = psum.tile([1, B], f32, tag="stats", bufs=2, name="ps_sxx")
            sq_ts = []
            for jt in range(JT):
                if step == 0:
                    nc.scalar.activation(xc[:, jt, :], pxs[jt][:], Act.Identity,
                                         bias=fcb_sb[:, jt:jt + 1], scale=1.0)
                else:
                    nc.scalar.copy(xc[:, jt, :], pxs[jt][:])
                sqt = work.tile([128, B], f32r, tag="sq", name="sqt")
                nc.gpsimd.tensor_tensor(sqt[:], F(xc[:, jt, :]),
                                        F(xc[:, jt, :]), Alu.mult)
                sq_ts.append(sqt)
            for jt in range(JT):
                nc.tensor.matmul(ps_sx[:], R(ones[:, 0:1]), R(xc[:, jt, :]),
                                 start=(jt == 0), stop=(jt == JT - 1))
            for jt in range(JT):
                nc.tensor.matmul(ps_sxx[:], R(ones[:, 0:1]), R(sq_ts[jt][:]),
                                 start=(jt == 0), stop=(jt == JT - 1))
            if skip_stats:
                inv = work.tile([1, B], f32r, tag="v1", bufs=8, name="inv")
                cmu = work.tile([1, B], f32r, tag="v1", bufs=8, name="cmu")
                nc.vector.tensor_copy(inv[:], ps_sx[:])
                nc.vector.tensor_copy(cmu[:], ps_sxx[:])
            else:
                sx_sb = work.tile([1, B], f32r, tag="v1", bufs=8, name="sx_sb")
                sxx_sb = work.tile([1, B], f32r, tag="v1", bufs=8, name="sxx_sb")
                nc.vector.tensor_copy(sx_sb[:], ps_sx[:])
                nc.vector.tensor_copy(sxx_sb[:], ps_sxx[:])
                st_in = dram.tile([2, B], f32r, tag="stin", name="st_in")
                st_out = dram.tile([2 * NCORES, B], f32r, tag="stout",
                                   addr_space="Shared", name="st_out")
                nc.sync.dma_start(st_in[0:1, :], sx_sb[:])
                nc.sync.dma_start(st_in[1:2, :], sxx_sb[:])
                nc.gpsimd.collective_compute(
                    "AllGather", Alu.bypass, replica_groups=RG,
                    ins=[st_in[:].opt()], outs=[st_out[:].opt()])
                sxall = work.tile([NCORES, B], f32r, tag="sall")
                sxxall = work.tile([NCORES, B], f32r, tag="sall")
                sov = st_out.rearrange("(r two) b -> two r b", two=2)
                nc.sync.dma_start(sxall[:], sov[0])
                nc.sync.dma_start(sxxall[:], sov[1])
                # reduce the 8 per-core partials on the PE (K=8 ones-matmul)
                ps_rsx = psum.tile([1, B], f32, tag="stats", bufs=2, name="ps_rsx")
                ps_rsxx = psum.tile([1, B], f32, tag="stats", bufs=2, name="ps_rsxx")
                nc.tensor.matmul(ps_rsx[:], R(ones[0:NCORES, 0:1]), R(sxall[:]),
                                 start=True, stop=True)
                nc.tensor.matmul(ps_rsxx[:], R(ones[0:NCORES, 0:1]), R(sxxall[:]),
                                 start=True, stop=True)
                # finalize: a = 1/sqrt(var+eps), c = mu*a
                mu = work.tile([1, B], f32, tag="v1", bufs=8, name="mu")
                ex2 = work.tile([1, B], f32, tag="v1", bufs=8, name="ex2")
                var = work.tile([1, B], f32, tag="v1", bufs=8, name="var")
                std = work.tile([1, B], f32, tag="v1", bufs=8, name="std")
                inv = work.tile([1, B], f32r, tag="v1", bufs=8, name="inv")
                cmu = work.tile([1, B], f32r, tag="v1", bufs=8, name="cmu")
                nc.vector.tensor_scalar_mul(mu[:], ps_rsx[:], 1.0 / N)
                nc.vector.tensor_scalar_mul(ex2[:], ps_rsxx[:], 1.0 / N)
                nc.vector.tensor_tensor(var[:], mu[:], mu[:], Alu.mult)
                nc.vector.tensor_tensor(var[:], ex2[:], var[:], Alu.subtract)
                nc.vector.tensor_scalar_add(var[:], var[:], EPS)
                nc.scalar.activation(std[:], var[:], Act.Sqrt)
                nc.vector.reciprocal(inv[:], std[:])
                nc.vector.tensor_tensor(cmu[:], mu[:], F(inv[:]), Alu.mult)
            # broadcast along partitions via K=1 matmul
            bcA = psum.tile([128, B], f32, tag="bc", bufs=2, name="bcA")
            bcC = psum.tile([128, B], f32, tag="bc", bufs=2, name="bcC")
            nc.tensor.matmul(bcA[:], R(ones[0:1, :]), R(inv[:]),
                             start=True, stop=True)
            nc.tensor.matmul(bcC[:], R(ones[0:1, :]), R(cmu[:]),
                             start=True, stop=True)
            # neuron update per j-tile
            for jt in range(JT if not skip_elem else 0):
                t1 = work.tile([128, B], f32, tag="t1", bufs=3, name="t1")
                nc.vector.tensor_tensor(t1[:], F(xc[:, jt, :]), bcA[:],
                                        Alu.mult)
                nc.vector.tensor_tensor(t1[:], t1[:], bcC[:], Alu.subtract)
                aq = work.tile([128, B], f32, tag="aq", name="aq")
                nc.gpsimd.tensor_scalar(aq[:], q_sb[:, jt, :], LEAK,
                                        cb_sb[:, jt:jt + 1], Alu.mult, Alu.add)
                g = work.tile([128, B], f32, tag="g", name="g")
                nc.vector.scalar_tensor_tensor(g[:], t1[:],
                                               lng_sb[:, jt:jt + 1], aq[:],
                                               Alu.mult, Alu.add)
                nc.gpsimd.tensor_scalar_min(q_sb[:, jt, :], g[:], 0.0)
                sg = work.tile([128, B], f32, tag="sg", name="sg")
                nc.scalar.activation(sg[:], g[:], Act.Sigmoid)
                # spikes = relu(g) * sigmoid(g)  (== silu(g) * (g > 0))
                nc.vector.scalar_tensor_tensor(s_sb[:, jt, :], g[:], 0.0,
                                               sg[:], Alu.max, Alu.mult)

        def spikes_ag():
            sp_in = dram.tile([JS, B], f32r, tag="spin", name="sp_in")
            ag_out = dram.tile([N, B], f32r, tag="agout", addr_space="Shared",
                               name="ag_out")
            nc.sync.dma_start(
                sp_in.rearrange("(t p) b -> p t b", p=128), s_sb[:])
            nc.gpsimd.collective_compute(
                "AllGather", Alu.bypass, replica_groups=RG,
                ins=[sp_in[:].opt()], outs=[ag_out[:].opt()])
            return ag_out

        def load_spT(ag_out):
            v = ag_out.rearrange("(c p) b -> p c b", p=128)
            for ch in range(4):
                nc.sync.dma_start(spT[:, ch * 8:(ch + 1) * 8, :],
                                  v[:, ch * 8:(ch + 1) * 8, :])

        def recurrent_gemm():
            pxs = [psum.tile([128, B], f32, tag="gemm", name=f"px{j}")
                   for j in range(JT)]
            for kc in range(KC):
                for jt in range(JT):
                    nc.tensor.matmul(pxs[jt][:],
                                     R(w_sb[:, kc, ts(jt, 128)]),
                                     R(spT[:, kc, :]),
                                     start=(kc == 0), stop=(kc == KC - 1))
            return pxs

        for step in range(steps):
            stats_and_elementwise(step, psum_xs)
            ag = spikes_ag()
            if step < steps - 1:
                if not skip_spike_ag or step == 0:
                    load_spT(ag)
                if not skip_gemm:
                    psum_xs = recurrent_gemm()

        # ---------------- classifier ----------------
        load_spT(ag)
        ps_cls = psum.tile([10, B], f32, tag="bc", bufs=2, name="ps_cls")
        for kc in range(KC):
            nc.tensor.matmul(ps_cls[:], R(clsw_sb[:, kc, :]), R(spT[:, kc, :]),
                             start=(kc == 0), stop=(kc == KC - 1))
        out_sb = work.tile([10, B], f32, tag="outsb")
        nc.scalar.activation(out_sb[:], ps_cls[:], Act.Identity,
                             bias=clsb_sb[:], scale=1.0)
        nc.sync.dma_start(out_d[:], out_sb[:])

    nc.compile()
    return nc


def _round_f32r(a):
    """Round fp32 to fp32r (11-bit mantissa, round-to-nearest-even)."""
    b = np.ascontiguousarray(a, np.float32).view(np.uint32).astype(np.uint64)
    lsb = (b >> 12) & 1
    out = ((b + 0x7FF + lsb) & 0xFFFFF000).astype(np.uint32)
    return out.view(np.float32)


def _host_prep(inputs):
    """Shard + lay out the full inputs for the 8 cores."""
    x = np.asarray(inputs["x"], np.float32)
    xpad = np.zeros((B, 30, 30), np.float32)
    xpad[:, 1:29, 1:29] = x[:, 0]
    w1t = np.ascontiguousarray(
        np.asarray(inputs["conv1_w"], np.float32).reshape(32, 9).T)
    w2col = _round_f32r(np.ascontiguousarray(
        np.asarray(inputs["conv2_w"], np.float32).transpose(2, 3, 1, 0)
        .reshape(3, 96, 64)))
    bn1 = np.stack([inputs["bn1_g"], inputs["bn1_b"],
                    inputs["bn1_m"], inputs["bn1_v"]]).astype(np.float32)
    bn2 = np.stack([inputs["bn2_g"], inputs["bn2_b"],
                    inputs["bn2_m"], inputs["bn2_v"]]).astype(np.float32)
    fc_w = np.asarray(inputs["fc_w"], np.float32)
    fc_b = np.asarray(inputs["fc_b"], np.float32)
    rec_w = np.asarray(inputs["rec_w"], np.float32)
    clswT = _round_f32r(np.ascontiguousarray(
        np.asarray(inputs["cls_w"], np.float32).T))
    clsb = np.asarray(inputs["cls_b"], np.float32)
    lng = np.asarray(inputs["ln_g"], np.float32)
    lnb = np.asarray(inputs["ln_b"], np.float32)
    thr = np.asarray(inputs["threshold"], np.float32)
    intr = np.asarray(inputs["intrinsic"], np.float32)

    in_maps = []
    for r in range(NCORES):
        js = slice(r * JS, (r + 1) * JS)
        in_maps.append(dict(
            xpad=np.ascontiguousarray(xpad[r * BS:(r + 1) * BS]),
            w1t=w1t, w2col=w2col, bn1=bn1, bn2=bn2,
            fcwT=_round_f32r(np.ascontiguousarray(fc_w[js].T)),
            fcb=np.ascontiguousarray(fc_b[js]),
            wT=np.ascontiguousarray(rec_w[js].T),
            lng=np.ascontiguousarray(lng[js]),
            lnb=np.ascontiguousarray(lnb[js]),
            thr=np.ascontiguousarray(thr[js]),
            intr=np.ascontiguousarray(intr[js]),
            clswT=clswT, clsb=clsb,
        ))
    return in_maps


def kernel(**inputs) -> np.ndarray:
    from concourse import bass_utils

    if "nc" not in _PROGRAM_CACHE:
        _PROGRAM_CACHE["nc"] = _build_program()
    nc = _PROGRAM_CACHE["nc"]

    in_maps = _host_prep(inputs)
    res = bass_utils.run_bass_kernel_spmd(
        nc, in_maps, core_ids=list(range(NCORES)))
    _PROGRAM_CACHE["last_results"] = res
    out = res.results[0]["out"]
    return np.ascontiguousarray(out.T.astype(np.float32))

